# revision 5
# baseline (speedup 1.0000x reference)
"""Self-contained Trainium2 Bass kernel for the GCN encoder problem.

4x GCNConv (relu on first 3) + linear skip + global mean pool over graphs.

Strategy (8 NeuronCores):
  - Nodes are partitioned into 8 equal contiguous shards (dst ownership).
  - Self-loops are appended as explicit edges; symmetric norm folded into a
    per-edge scalar, applied on-chip to gathered rows.
  - GCNConv is computed aggregation-first (aggregate x, then @W), so only
    128-wide tables are ever gathered.
  - Per layer: every core holds the full activation table in HBM, gathers
    rows for its shard's edges with dma_gather (int16 chunked indices),
    segment-sums them per 128-node window with selection-matrix matmuls on
    the PE, applies the dense layer weights locally, and contributes its
    shard's new activations to an AllGather that rebuilds the table.
  - Mean-pool partials are computed per-core with selection matmuls;
    host sums the 8 partials and divides by graph sizes.
"""
import numpy as np

import concourse.bass as bass
import concourse.bacc as bacc
import concourse.mybir as mybir
import concourse.tile as tile
from concourse.bass_utils import run_bass_kernel_spmd
from concourse.library_config import mlp as mlp_lib

P = 128
NC = 8


# ---------------------------------------------------------------- host prep
def _prepare(x, edge_index, batch, N, G):
    SH = N // NC
    SHP = -(-SH // P) * P
    NW = SHP // P
    CH = SHP * 2                      # rows per chunk (2 shards) < 32768
    NCHUNK = 4
    assert CH < 32768 and SHP * NC == CH * NCHUNK

    src = edge_index[0].astype(np.int64)
    dst = edge_index[1].astype(np.int64)
    E = src.shape[0]
    deg = np.bincount(dst, minlength=N).astype(np.float32) + 1.0
    dis = (1.0 / np.sqrt(deg)).astype(np.float32)

    src_all = np.concatenate([src, np.arange(N, dtype=np.int64)])
    dst_all = np.concatenate([dst, np.arange(N, dtype=np.int64)])
    norm_all = np.concatenate([dis[src] * dis[dst], dis * dis]).astype(np.float32)

    srow = (src_all // SH) * SHP + (src_all % SH)
    eshard = dst_all // SH
    lrow = dst_all - eshard * SH
    ewin = lrow // P
    edloc = (lrow % P).astype(np.float32)
    echunk = srow // CH
    ecidx = (srow % CH).astype(np.int64)

    counts = np.zeros((NC, NCHUNK, NW), dtype=np.int64)
    for s in range(NC):
        m = eshard == s
        np.add.at(counts[s], (echunk[m], ewin[m]), 1)
    tiles = np.maximum(-(-counts.max(axis=0) // P), 1)   # [NCHUNK, NW]
    slots = tiles * P

    slot_off = np.zeros((NCHUNK, NW), dtype=np.int64)
    off = 0
    for c in range(NCHUNK):
        for w in range(NW):
            slot_off[c, w] = off
            off += slots[c, w]
    tot_slots = off
    tot_tiles = tot_slots // P

    cidx16 = np.zeros((NC, tot_slots), dtype=np.int16)
    dstloc = np.zeros((NC, tot_slots), dtype=np.float32)
    normv = np.zeros((NC, tot_slots), dtype=np.float32)
    for s in range(NC):
        m = eshard == s
        ec, ew = echunk[m], ewin[m]
        order = np.lexsort((ew, ec))
        ci = ecidx[m][order]
        dl = edloc[m][order]
        nv = norm_all[m][order]
        pos = 0
        for c in range(NCHUNK):
            for w in range(NW):
                n = counts[s, c, w]
                o = slot_off[c, w]
                cidx16[s, o:o+n] = ci[pos:pos+n].astype(np.int16)
                dstloc[s, o:o+n] = dl[pos:pos+n]
                normv[s, o:o+n] = nv[pos:pos+n]
                pos += n
        assert pos == int(m.sum())

    F = x.shape[1]
    NPAD = SHP * NC
    xt = np.zeros((NPAD, F), dtype=np.float32)
    ids = np.arange(N)
    xt[(ids // SH) * SHP + ids % SH] = x

    batchf = np.full((NC, SHP), -1.0, dtype=np.float32)
    for s in range(NC):
        batchf[s, :SH] = batch[s*SH:(s+1)*SH].astype(np.float32)
    cnts = np.bincount(batch, minlength=G).astype(np.float32)

    # wrapped idx layout for dma_gather: slot i -> [i%16 + 16k, i//16]
    cidx_w = np.zeros((NC, P, tot_slots // 16), dtype=np.int16)
    for s in range(NC):
        w16 = cidx16[s].reshape(tot_slots // 16, 16).T   # [16, slots/16]
        cidx_w[s] = np.tile(w16, (8, 1))

    # slot-tile layout for dstloc/norm: edge slot t*128+p -> [p, t]
    dst_T = dstloc.reshape(NC, tot_tiles, P).transpose(0, 2, 1).copy()
    nrm_T = normv.reshape(NC, tot_tiles, P).transpose(0, 2, 1).copy()

    return dict(
        SH=SH, SHP=SHP, NW=NW, CH=CH, NCHUNK=NCHUNK, NPAD=NPAD,
        dis=dis, xt=xt, cidx_w=cidx_w, dst_T=dst_T, nrm_T=nrm_T,
        batchf=batchf, cnts=cnts, tiles=tiles, slot_off=slot_off,
        tot_slots=tot_slots, tot_tiles=tot_tiles,
    )


# ---------------------------------------------------------------- device build
def _build(meta, F, H, G, GW=2):
    """Build the SPMD Bass program (same structure for all cores)."""
    SHP, NW, CH, NCHUNK, NPAD = (
        meta["SHP"], meta["NW"], meta["CH"], meta["NCHUNK"], meta["NPAD"])
    tiles, slot_off = meta["tiles"], meta["slot_off"]
    tot_slots, tot_tiles = meta["tot_slots"], meta["tot_tiles"]
    H2 = 2 * H
    f32 = mybir.dt.float32

    nc = bacc.Bacc("TRN2", target_bir_lowering=False, debug=False, num_devices=NC)

    xt = nc.dram_tensor("xt", [NPAD, F], f32, kind="ExternalInput")
    xtl = nc.dram_tensor("xtl", [P, SHP], f32, kind="ExternalInput")
    wts = nc.dram_tensor("wts", [P, 7 * P], f32, kind="ExternalInput")
    bia = nc.dram_tensor("bia", [P, 4], f32, kind="ExternalInput")
    iot = nc.dram_tensor("iot", [P, 1, P], f32, kind="ExternalInput")
    idn = nc.dram_tensor("idn", [P, P], f32, kind="ExternalInput")
    cix = nc.dram_tensor("cix", [P, tot_slots // 16], mybir.dt.int16,
                         kind="ExternalInput")
    dlo = nc.dram_tensor("dlo", [P, tot_tiles], f32, kind="ExternalInput")
    nrm = nc.dram_tensor("nrm", [P, tot_tiles], f32, kind="ExternalInput")
    bat = nc.dram_tensor("bat", [P, NW], f32, kind="ExternalInput")
    pooled = nc.dram_tensor("pooled", [G, H], f32, kind="ExternalOutput")

    tabs = [
        nc.dram_tensor(f"tab{k}", [NPAD, H], f32, kind="Internal",
                       addr_space="Shared")
        for k in (2, 3, 4)
    ]

    NG = NW // GW  # gather groups
    assert NG * GW == NW

    with tile.TileContext(nc, num_cores=NC) as tc:
        with (
            tc.tile_pool(name="res", bufs=1) as res,
            tc.tile_pool(name="gp", bufs=2) as gp,
            tc.tile_pool(name="sp", bufs=4) as sp,
            tc.tile_pool(name="wk", bufs=2) as wk,
            tc.tile_pool(name="pa", bufs=2, space="PSUM") as pa,
            tc.tile_pool(name="pb", bufs=2, space="PSUM") as pb,
            tc.tile_pool(name="pcp", bufs=2, space="PSUM") as pcp,
            tc.tile_pool(name="pq", bufs=1, space="PSUM") as pq,
            tc.tile_pool(name="dr", bufs=1, space="DRAM") as dr,
        ):
            nc.gpsimd.load_library(mlp_lib)

            # resident loads
            cix_sb = res.tile([P, tot_slots // 16], mybir.dt.int16)
            nc.sync.dma_start(cix_sb[:], cix[:])
            dlo_sb = res.tile([P, tot_tiles], f32)
            nc.sync.dma_start(dlo_sb[:], dlo[:])
            nrm_sb = res.tile([P, tot_tiles], f32)
            nc.sync.dma_start(nrm_sb[:], nrm[:])
            wts_sb = res.tile([P, 7 * P], f32)
            nc.sync.dma_start(wts_sb[:], wts[:])
            bia_sb = res.tile([P, 4], f32)
            nc.sync.dma_start(bia_sb[:], bia[:])
            iot_sb = res.tile([P, 1, P], f32)
            nc.sync.dma_start(iot_sb[:], iot[:])
            idn_sb = res.tile([P, P], f32)
            nc.sync.dma_start(idn_sb[:], idn[:])
            bat_sb = res.tile([P, NW], f32)
            nc.sync.dma_start(bat_sb[:], bat[:])
            acc_sb = res.tile([G, H], f32)
            nc.vector.memset(acc_sb[:], 0.0)

            W = [wts_sb[:, i*P:(i+1)*P] for i in range(7)]
            W1, W2, W3a, W3b, W4a, W4b, Wl = W

            bounces = []
            for k in range(3):
                bn = dr.tile([SHP, H], f32, tag=f"bn{k}", name=f"bounce{k}")
                bounces.append(bn)

            def layer(k):
                """k in 0..3 -> conv layers 1..4."""
                table = xt if k == 0 else tabs[k - 1]
                for g in range(NG):
                    w0 = g * GW
                    gouts = []
                    for c in range(NCHUNK):
                        a = int(slot_off[c, w0])
                        ns = int(slots_rng(c, w0, GW))
                        gt = gp.tile([P, ns // P, P], f32, tag=f"g{c}")
                        nc.gpsimd.dma_gather(
                            gt[:], table[c*CH:(c+1)*CH, :],
                            cix_sb[:, a//16:(a + ns)//16], ns, ns, H if k else F,
                        )
                        # fold per-edge norm into gathered rows (in place)
                        ta = a // P
                        nc.vector.tensor_tensor(
                            out=gt[:], in0=gt[:],
                            in1=nrm_sb[:, ta:ta + ns//P].to_broadcast(
                                [P, ns // P, P]),
                            op=mybir.AluOpType.mult,
                        )
                        gouts.append((gt, ta))
                    for wi in range(GW):
                        w = w0 + wi
                        u = pa.tile([P, P], f32, tag="u")
                        first = True
                        nmm = int(tiles[:, w].sum()) + (1 if k == 3 else 0)
                        mmi = 0
                        for c in range(NCHUNK):
                            gt, ta = gouts[c]
                            t0 = int(slot_off[c, w]) // P - ta
                            for t in range(int(tiles[c, w])):
                                # build S tile [P, P]
                                st = sp.tile([P, P], f32, tag="s")
                                nc.vector.tensor_tensor(
                                    out=st[:],
                                    in0=dlo_sb[:, ta + t0 + t:ta + t0 + t + 1]
                                        .to_broadcast([P, P]),
                                    in1=iot_sb[:, 0, :],
                                    op=mybir.AluOpType.is_equal,
                                )
                                nc.tensor.matmul(
                                    u[:], lhsT=gt[:, t0 + t, :].opt(),
                                    rhs=st[:],
                                    start=first, stop=(mmi == nmm - 1),
                                )
                                first = False
                                mmi += 1
                        # epilogues
                        if k == 0 or k == 1:
                            usb = wk.tile([P, P], f32, tag="usb")
                            nc.vector.tensor_copy(usb[:], u[:])
                            y = pb.tile([P, P], f32, tag="y")
                            nc.tensor.matmul(y[:], lhsT=W[k], rhs=usb[:],
                                             start=True, stop=True)
                            xk = wk.tile([P, P], f32, tag="xk")
                            nc.scalar.activation(
                                xk[:], y[:], mybir.ActivationFunctionType.Relu,
                                bias=bia_sb[:, k:k+1])
                            xp = pcp.tile([P, P], f32, tag="t")
                            nc.tensor.transpose(xp[:], xk[:], idn_sb[:])
                            xo = wk.tile([P, P], f32, tag="xo")
                            nc.vector.tensor_copy(xo[:], xp[:])
                            nc.sync.dma_start(
                                bounces[k][w*P:(w+1)*P, :], xo[:])
                        elif k == 2:
                            usb = wk.tile([P, P], f32, tag="usb")
                            nc.vector.tensor_copy(usb[:], u[:])
                            v = pq.tile([P, P], f32, tag="v")
                            for h in range(2):
                                y = pb.tile([P, P], f32, tag="y")
                                nc.tensor.matmul(
                                    y[:], lhsT=(W3a, W3b)[h], rhs=usb[:],
                                    start=True, stop=True)
                                x3 = wk.tile([P, P], f32, tag=f"x3{h}")
                                nc.scalar.activation(
                                    x3[:], y[:],
                                    mybir.ActivationFunctionType.Relu,
                                    bias=bia_sb[:, 2+h:3+h])
                                nc.tensor.matmul(
                                    v[:], lhsT=(W4a, W4b)[h], rhs=x3[:],
                                    start=(h == 0), stop=(h == 1))
                            vsb = wk.tile([P, P], f32, tag="usb")
                            nc.vector.tensor_copy(vsb[:], v[:])
                            vp = pcp.tile([P, P], f32, tag="t")
                            nc.tensor.transpose(vp[:], vsb[:], idn_sb[:])
                            vo = wk.tile([P, P], f32, tag="xo")
                            nc.vector.tensor_copy(vo[:], vp[:])
                            nc.sync.dma_start(
                                bounces[2][w*P:(w+1)*P, :], vo[:])
                        else:
                            # k == 3: +skip into u, then pool
                            xtw = wk.tile([P, P], f32, tag="xtw")
                            nc.sync.dma_start(xtw[:], xtl[:, w*P:(w+1)*P])
                            nc.tensor.matmul(u[:], lhsT=Wl, rhs=xtw[:],
                                             start=False, stop=True)
                            osb = wk.tile([P, P], f32, tag="usb")
                            nc.vector.tensor_copy(osb[:], u[:])
                            op_ = pcp.tile([P, P], f32, tag="t")
                            nc.tensor.transpose(op_[:], osb[:], idn_sb[:])
                            oo = wk.tile([P, P], f32, tag="xo")
                            nc.vector.tensor_copy(oo[:], op_[:])
                            spl = wk.tile([P, G], f32, tag="spl")
                            nc.vector.tensor_tensor(
                                out=spl[:],
                                in0=bat_sb[:, w:w+1].to_broadcast([P, G]),
                                in1=iot_sb[:, 0, :G],
                                op=mybir.AluOpType.is_equal,
                            )
                            pp = pq.tile([G, H], f32, tag="pool")
                            nc.tensor.matmul(pp[:], lhsT=spl[:], rhs=oo[:],
                                             start=True, stop=True)
                            nc.vector.tensor_add(acc_sb[:], acc_sb[:], pp[:])

                if k < 3:
                    nc.gpsimd.collective_compute(
                        "AllGather", mybir.AluOpType.bypass,
                        replica_groups=[list(range(NC))],
                        ins=[bounces[k][:]], outs=[tabs[k][:]],
                    )

            def slots_rng(c, w0, gw):
                return int(slot_off[c, w0 + gw - 1] + tiles[c, w0 + gw - 1] * P
                           - slot_off[c, w0])

            for k in range(4):
                layer(k)
            nc.sync.dma_start(pooled[:], acc_sb[:])

    nc.compile()
    return nc


# ---------------------------------------------------------------- entry point
def _prep_all(x, edge_index, batch, W1, b1, W2, b2, W3, b3, W4, b4, Wl, bl,
              N, G, GW=2):
    F = x.shape[1]
    H = W1.shape[1]
    meta = _prepare(np.asarray(x), np.asarray(edge_index), np.asarray(batch),
                    N, G)
    nc = _build(meta, F, H, G, GW=GW)

    wcat = np.concatenate(
        [W1, W2, W3[:, :H], W3[:, H:], W4[:H, :], W4[H:, :], Wl],
        axis=1).astype(np.float32)                       # [F, 7H]
    bcat = np.stack([b1, b2, b3[:H], b3[H:]], axis=1).astype(np.float32)
    iota = np.tile(np.arange(P, dtype=np.float32)[None, None, :], (P, 1, 1))
    ident = np.eye(P, dtype=np.float32)

    SH, SHP = meta["SH"], meta["SHP"]
    in_maps = []
    for s in range(NC):
        xl = np.zeros((SHP, F), dtype=np.float32)
        xl[:SH] = np.asarray(x)[s*SH:(s+1)*SH]
        in_maps.append({
            "xt": meta["xt"],
            "xtl": np.ascontiguousarray(xl.T),
            "wts": wcat, "bia": bcat, "iot": iota, "idn": ident,
            "cix": meta["cidx_w"][s], "dlo": meta["dst_T"][s],
            "nrm": meta["nrm_T"][s], "bat": meta["batchf"][s].reshape(
                meta["NW"], P).T.copy(),
        })

    return nc, in_maps, meta


def _run(x, edge_index, batch, W1, b1, W2, b2, W3, b3, W4, b4, Wl, bl,
         N, G, GW=2, trace=False):
    H = W1.shape[1]
    nc, in_maps, meta = _prep_all(x, edge_index, batch, W1, b1, W2, b2,
                                  W3, b3, W4, b4, Wl, bl, N, G, GW=GW)
    res = run_bass_kernel_spmd(nc, in_maps, core_ids=list(range(NC)),
                               trace=trace)
    total = np.zeros((G, H), dtype=np.float64)
    for s in range(NC):
        total += res.results[s]["pooled"].astype(np.float64)
    cnts = meta["cnts"]
    total += cnts[:, None].astype(np.float64) * (b4 + bl)[None, :]
    out = (total / np.maximum(cnts, 1.0)[:, None]).astype(np.float32)
    return out, res


def kernel(x, edge_index, batch, W1, b1, W2, b2, W3, b3, W4, b4, Wl, bl):
    out, _ = _run(x, edge_index, batch, W1, b1, W2, b2, W3, b3, W4, b4,
                  Wl, bl, N=100000, G=64)
    return out
